# revision 27
# baseline (speedup 1.0000x reference)
import sys

if "/opt/trn_rl_repo" not in sys.path:
    sys.path.insert(0, "/opt/trn_rl_repo")

import numpy as np

import concourse.bacc as bacc
import concourse.tile as tile
from concourse import bass_utils, mybir
from concourse.bass import ts
from concourse.masks import make_identity

F32 = mybir.dt.float32
BF16 = mybir.dt.bfloat16
EXP = mybir.ActivationFunctionType.Exp


# nn_MultiHeadedAttention: B=2, S=2048, D=1024, H=16, DH=64.
# 16 heads over 8 cores (2 heads/core = 128 features). QKV column-parallel,
# out-projection row-parallel, host sums the 8 partial outputs.
#
# Schedule: phase-1 chunk 0 runs first, then attention for query-chunk 0
# starts immediately while phase-1 chunks 1..7 are interleaved in small
# units between attention j-iterations (just-in-time for the key tiles
# they produce). Batch-0 out-projections are deferred into batch-1's
# ACT-slack stretches so the scalar engine (softmax exp, the pacing
# engine at ~1.1us per j) never starves. DMA triggers are minimized --
# the sync engine serializes them at ~0.6us apiece.
B, S, D, H = 2, 2048, 1024, 16
DH = D // H
NC = 8
T = B * S                  # 4096 tokens
NCHUNK = T // 512          # 8 token chunks of 512
KCH = D // 128             # 8 contraction chunks
NJ = S // 128              # 16 key tiles per batch
QC = S // 512              # 4 query chunks per batch

_CACHE = {}


def _build():
    if "nc" in _CACHE:
        return _CACHE["nc"]

    nc = bacc.Bacc("TRN2", target_bir_lowering=False, debug=False,
                   enable_asserts=True, num_devices=NC)

    xT = nc.dram_tensor("xT", [D, T], BF16, kind="ExternalInput").ap()
    wq = nc.dram_tensor("wq", [D, 128], BF16, kind="ExternalInput").ap()
    wk = nc.dram_tensor("wk", [D, 128], BF16, kind="ExternalInput").ap()
    wv = nc.dram_tensor("wv", [D, 128], BF16, kind="ExternalInput").ap()
    wo = nc.dram_tensor("wo", [128, D], BF16, kind="ExternalInput").ap()
    bqk = nc.dram_tensor("bqk", [1, 256], F32, kind="ExternalInput").ap()
    outT = nc.dram_tensor("outT", [D, T], BF16, kind="ExternalOutput").ap()

    with tile.TileContext(nc) as tc:
        with (
            tc.tile_pool(name="wpool", bufs=1) as wpool,
            tc.tile_pool(name="qk", bufs=1) as qk_pool,
            tc.tile_pool(name="vtm", bufs=1) as vtm_pool,
            tc.tile_pool(name="on", bufs=1) as on_pool,
            tc.tile_pool(name="xin", bufs=4) as xin_pool,
            tc.tile_pool(name="vst", bufs=2) as vst_pool,
            tc.tile_pool(name="epool", bufs=6) as epool,
            tc.tile_pool(name="npool", bufs=2) as npool,
            tc.tile_pool(name="ostage", bufs=2) as ostage_pool,
            # PSUM budget (8 banks):
            #   psA 2x[128,1024]f32 = 4 banks (scores double-buffer)
            #   psO 1x[65,1024]f32  = 2 banks (o accumulator)
            #   psM 2x 1-bank tiles = 2 banks (q/k/v proj, v transposes,
            #       out-projection -- phase-1 is bank-serial: q releases
            #       its bank before v needs it)
            tc.tile_pool(name="psA", bufs=2, space="PSUM") as psA,
            tc.tile_pool(name="psO", bufs=1, space="PSUM") as psO,
            tc.tile_pool(name="psM", bufs=2, space="PSUM") as psM,
        ):
            # ---- weights / constants ----
            # wq/wk are split into halves (separate tiles, separate DMA
            # queues): per-queue DMA bandwidth is ~50-100 GB/s, and the
            # first projection matmul is gated on the first weight slice.
            wq_h = [wpool.tile([128, D // 2], BF16, name=f"wq{h}")
                    for h in range(2)]
            wk_h = [wpool.tile([128, D // 2], BF16, name=f"wk{h}")
                    for h in range(2)]
            wv_h = [wpool.tile([128, D // 2], BF16, name=f"wv{h}")
                    for h in range(2)]
            wo_sb = wpool.tile([128, D], BF16)

            def wslice(wh, k):
                return wh[k // 4][:, ts(k % 4, 128)]
            bq_sb = wpool.tile([128, 1], F32)
            bk_sb = wpool.tile([128, 1], F32)
            ident = wpool.tile([128, 128], BF16)

            # qn/kn/on persistent activations
            qn = [qk_pool.tile([128, 512], BF16, name=f"qn{n}")
                  for n in range(NCHUNK)]
            kn = [qk_pool.tile([128, 512], BF16, name=f"kn{n}")
                  for n in range(NCHUNK)]
            on = [on_pool.tile([128, 512], BF16, name=f"on{n}")
                  for n in range(NCHUNK)]
            v_tm = {}
            for hh in range(2):
                for J in range(2 * NJ):
                    v_tm[(hh, J)] = vtm_pool.tile(
                        [128, 128], BF16, name=f"vtm{hh}_{J}")

            # ---- phase 1 units for chunk n (emitted piecemeal) ----
            # x for a chunk loads as two tiles (two DMA triggers) so the
            # first projection matmuls can start on the first half.
            def make_ph1(n, nsplit=2):
                st = {}
                kper = KCH // nsplit

                def u_dma():
                    st["xp"] = []
                    for h in range(nsplit):
                        xh = xin_pool.tile([128, kper * 512], BF16, tag="xp",
                                           name=f"xp{n}_{h}")
                        st["xp"].append(xh)
                        nc.sync.dma_start(
                            xh[:],
                            xT[h * kper * 128:(h + 1) * kper * 128,
                               ts(n, 512)].rearrange(
                                "(k p) t -> p k t", p=128))

                def xs(k):
                    return st["xp"][k // kper][:, ts(k % kper, 512)]

                def u_q():
                    st["q_ps"] = psM.tile([128, 512], F32, tag="M",
                                          name=f"qps{n}")
                    for k in range(KCH):
                        nc.tensor.matmul(st["q_ps"][:], wslice(wq_h, k),
                                         xs(k),
                                         start=(k == 0), stop=(k == KCH - 1))

                def u_k():
                    st["k_ps"] = psM.tile([128, 512], F32, tag="M",
                                          name=f"kps{n}")
                    for k in range(KCH):
                        nc.tensor.matmul(st["k_ps"][:], wslice(wk_h, k),
                                         xs(k),
                                         start=(k == 0), stop=(k == KCH - 1))
                    nc.vector.tensor_scalar_add(qn[n][:], st["q_ps"][:],
                                                bq_sb[:])

                def u_v():
                    st["v_ps"] = psM.tile([128, 512], F32, tag="M",
                                          name=f"vps{n}")
                    for k in range(KCH):
                        nc.tensor.matmul(st["v_ps"][:], wslice(wv_h, k),
                                         xs(k),
                                         start=(k == 0), stop=(k == KCH - 1))
                    nc.vector.tensor_scalar_add(kn[n][:], st["k_ps"][:],
                                                bk_sb[:])

                def u_t():
                    vst = vst_pool.tile([128, 512], BF16, tag="vst",
                                        name=f"vst{n}")
                    nc.vector.tensor_copy(vst[:], st["v_ps"][:])
                    for jj in range(4):
                        # one fused [128,128] transpose covers both heads
                        t_ps = psM.tile([128, 1024], BF16, tag="M",
                                        name=f"tps{n}_{jj}")
                        nc.tensor.transpose(t_ps[:, 0:128],
                                            vst[:, ts(jj, 128)], ident[:])
                        # ones column FIRST (sums land on o_ps
                        # partition 0, readable by reciprocal straight
                        # from PSUM); dh at columns 64-127 so the oc copy
                        # reads a 64-aligned partition range. Columns
                        # 1-63 are never read downstream.
                        for hh in range(2):
                            vt = v_tm[(hh, 4 * n + jj)]
                            nc.vector.tensor_copy(
                                vt[:, 64:128], t_ps[:, hh * 64:(hh + 1) * 64])
                            nc.vector.memset(vt[:, 0:1], 1.0)

                return [u_dma, u_q, u_k, u_v, u_t]

            # ---- normalization + out-projection ----
            nstate = {}

            def emit_norm_copy(n, o_ps):
                # sums row sits at o_ps partition 0 (ones column first in
                # v_tm), so reciprocal reads PSUM directly -- no staging
                # copy of the sums row.
                oc = npool.tile([64, 1024], F32, tag="oc", name=f"oc{n}")
                r_sb = npool.tile([1, 1024], F32, tag="r", name=f"r{n}")
                nc.vector.reciprocal_approx_fast(r_sb[:], o_ps[0:1, :])
                nc.vector.tensor_copy(oc[:], o_ps[64:128, :])
                nstate[n] = (oc, r_sb)

            def emit_norm_rest(n):
                oc, r_sb = nstate.pop(n)
                rb = npool.tile([64, 1024], F32, tag="rb", name=f"rb{n}")
                nc.gpsimd.partition_broadcast(rb[:], r_sb[0:1, :])
                for hh in range(2):
                    hs = slice(hh * 64, (hh + 1) * 64)
                    nc.vector.tensor_tensor(
                        out=on[n][hs, :], in0=oc[0:64, ts(hh, 512)],
                        in1=rb[:, ts(hh, 512)], op=mybir.AluOpType.mult)

            def emit_oproj(n):
                for g in range(2):
                    ost = ostage_pool.tile([128, 2048], BF16, tag="ost",
                                           name=f"ost{n}_{g}")
                    for mi in range(4):
                        m = g * 4 + mi
                        op_ps = psM.tile([128, 512], F32, tag="M",
                                         name=f"opps{n}_{m}")
                        nc.tensor.matmul(op_ps[:], wo_sb[:, ts(m, 128)],
                                         on[n][:], start=True, stop=True)
                        nc.vector.tensor_copy(ost[:, ts(mi, 512)],
                                              op_ps[:])
                    # one DMA trigger per 4 m-chunks
                    dst = outT[g * 512:(g + 1) * 512, ts(n, 512)]
                    nc.sync.dma_start(
                        dst.rearrange("(m p) t -> p m t", p=128),
                        ost[:])

            # ---- attention for one query chunk, with hooks ----
            def emit_attn(b, qc, hooks):
                n = b * QC + qc
                o_ps = psO.tile([128, 1024], F32, tag="O", name=f"ops{n}")
                e_prev = None
                for j in range(NJ):
                    s_ps = psA.tile([128, 1024], F32, tag="A",
                                    name=f"sps{n}_{j}")
                    for hh in range(2):
                        hs = slice(hh * 64, (hh + 1) * 64)
                        nc.tensor.matmul(
                            s_ps[:, ts(hh, 512)],
                            kn[b * QC + j // 4][hs, ts(j % 4, 128)],
                            qn[n][hs, :], start=True, stop=True)
                    e_sb = epool.tile([128, 1024], BF16, tag="e",
                                      name=f"e{n}_{j}")
                    nc.scalar.activation(e_sb[:], s_ps[:], EXP)
                    if j >= 1:
                        for hh in range(2):
                            nc.tensor.matmul(
                                o_ps[0:128, ts(hh, 512)],
                                v_tm[(hh, b * NJ + j - 1)][:],
                                e_prev[:, ts(hh, 512)],
                                start=(j - 1 == 0), stop=False)
                    e_prev = e_sb
                    for fn in hooks.get(j, ()):
                        fn()
                for hh in range(2):
                    nc.tensor.matmul(
                        o_ps[0:128, ts(hh, 512)],
                        v_tm[(hh, b * NJ + NJ - 1)][:],
                        e_prev[:, ts(hh, 512)],
                        start=False, stop=True)
                return (n, o_ps)

            # ---- emission schedule ----
            ph1 = [make_ph1(n) for n in range(NCHUNK)]

            # head: identity first (gpsimd, no DMA) so warm-up
            # matmuls can start immediately; then x quarters interleaved
            # with weight halves by first-use order. Biases arrive as one
            # contiguous [1,256] row (a [128,1] DMA costs ~3us of sync
            # descriptor generation) and are scattered to partitions with
            # a K=1 matmul.
            make_identity(nc, ident[:])
            nc.sync.dma_start(
                wq_h[0][:],
                wq[0:512, :].rearrange("(k p) f -> p k f", p=128))
            ph1[0][0]()          # chunk 0 x DMA (2 triggers)
            bqk_sb = wpool.tile([1, 256], F32)
            nc.sync.dma_start(bqk_sb[:], bqk[:])
            nc.sync.dma_start(
                wq_h[1][:],
                wq[512:1024, :].rearrange("(k p) f -> p k f", p=128))
            for h in range(2):
                nc.sync.dma_start(
                    wk_h[h][:],
                    wk[h * 512:(h + 1) * 512, :].rearrange(
                        "(k p) f -> p k f", p=128))
            for h in range(2):
                nc.sync.dma_start(
                    wv_h[h][:],
                    wv[h * 512:(h + 1) * 512, :].rearrange(
                        "(k p) f -> p k f", p=128))
            # Warm the PE's HAM clock gate during the initial DMA wait:
            # ~3.4us of sustained activity lifts the PE from 1.2 to
            # 2.4 GHz, so chunk 0's projections run at full rate.
            warm = [psM.tile([128, 512], F32, tag="M", name=f"warm{i}")
                    for i in range(2)]
            one_sb = wpool.tile([1, 1], F32)
            nc.vector.memset(one_sb[:], 1.0)
            for i in range(40):
                nc.tensor.matmul(warm[i % 2][:, 0:128], ident[:], ident[:],
                                 start=True, stop=True)
            ph1[0][1]()          # u_q
            # scatter biases [1,256] -> [128,1] each via K=1 matmuls
            # (emitted after u_q so the PE never idles waiting for bqk,
            # and before u_k whose bias-add reads bq_sb)
            nc.tensor.matmul(warm[1][:, 0:1], bqk_sb[0:1, 0:128],
                             one_sb[:], start=True, stop=True)
            nc.tensor.matmul(warm[1][:, 1:2], bqk_sb[0:1, 128:256],
                             one_sb[:], start=True, stop=True)
            nc.vector.tensor_copy(bq_sb[:], warm[1][:, 0:1])
            nc.vector.tensor_copy(bk_sb[:], warm[1][:, 1:2])
            ph1[0][2]()          # u_k
            nc.sync.dma_start(wo_sb[:], wo[:])
            # Warm the ACT exp table.
            dummy = wpool.tile([1, 2], F32)
            nc.vector.memset(dummy[:], 0.0)
            nc.scalar.activation(dummy[:], dummy[:], EXP)
            ph1[0][3]()          # u_v
            ph1[0][4]()          # u_t
            ph1[1][0]()          # chunk 1 DMA prefetch

            # stretch 1: qc0 carries ph1 chunks 1-3 (just-in-time for its
            # own key tiles)
            pending = emit_attn(0, 0, {
                0: [ph1[1][1]], 1: [ph1[1][2]], 2: [ph1[1][3]],
                3: [ph1[1][4], ph1[2][0]],
                4: [ph1[2][1]], 5: [ph1[2][2]], 6: [ph1[2][3]],
                7: [ph1[2][4], ph1[3][0]],
                8: [ph1[3][1]], 9: [ph1[3][2]], 10: [ph1[3][3]],
                11: [ph1[3][4]],
            })

            def stretch(b, qc, c=None, defer=None, cur=None, defer2=None):
                """One attention stretch: pending norm (copy at j0, rest
                at j5), optional phase-1 chunk c spread at js 1..12,
                optional deferred out-projections at j3/j12, current
                chunk's out-projection at j9."""
                nonlocal pending
                hooks = {}
                pn, po = pending
                hooks[0] = [lambda: emit_norm_copy(pn, po)]
                if defer is not None:
                    hooks.setdefault(3, []).append(
                        lambda: emit_oproj(defer))
                hooks.setdefault(5, []).append(lambda: emit_norm_rest(pn))
                if cur is not None:
                    hooks.setdefault(9, []).append(
                        lambda: emit_oproj(cur))
                if defer2 is not None:
                    hooks.setdefault(12, []).append(
                        lambda: emit_oproj(defer2))
                if c is not None:
                    hooks.setdefault(1, []).append(ph1[c][0])
                    hooks.setdefault(3, []).append(ph1[c][1])
                    hooks.setdefault(6, []).append(ph1[c][2])
                    hooks.setdefault(9, []).append(ph1[c][3])
                    hooks.setdefault(12, []).append(ph1[c][4])
                pending = emit_attn(b, qc, hooks)

            stretch(0, 1, c=4)
            stretch(0, 2, c=5)
            stretch(0, 3, c=6)
            stretch(1, 0, c=7)
            stretch(1, 1, defer=0, cur=4)
            stretch(1, 2, defer=1, cur=5)
            stretch(1, 3, defer=2, cur=6, defer2=3)

            # tail: chunk 7 norm + out-projection. The on[] multiplies
            # read o_ps straight from PSUM (no oc staging copy -- nothing
            # else needs the banks), and dummy matmuls keep the PE's HAM
            # clock warm through the ~4us norm chain so the final
            # out-projection runs at full rate.
            pn, po = pending          # chunk 7
            r_sb = npool.tile([1, 1024], F32, tag="r", name=f"r{pn}")
            nc.vector.reciprocal_approx_fast(r_sb[:], po[0:1, :])
            rb = npool.tile([64, 1024], F32, tag="rb", name=f"rb{pn}")
            nc.gpsimd.partition_broadcast(rb[:], r_sb[0:1, :])
            wtl = [psM.tile([128, 512], F32, tag="M", name=f"wtl{i}")
                   for i in range(2)]
            for i in range(12):
                nc.tensor.matmul(wtl[i % 2][:, 0:128], ident[:], ident[:],
                                 start=True, stop=True)
            for hh in range(2):
                hs = slice(hh * 64, (hh + 1) * 64)
                nc.vector.tensor_tensor(
                    out=on[pn][hs, :], in0=po[64:128, ts(hh, 512)],
                    in1=rb[:, ts(hh, 512)], op=mybir.AluOpType.mult)
            for i in range(12):
                nc.tensor.matmul(wtl[i % 2][:, 0:128], ident[:], ident[:],
                                 start=True, stop=True)
            emit_oproj(pn)

    nc.compile()
    _CACHE["nc"] = nc
    return nc


def _prep_in_maps(x, Wq, bq, Wk, bk, Wv, Wo):
    import ml_dtypes
    bf16 = ml_dtypes.bfloat16
    xT = np.ascontiguousarray(x.reshape(T, D).T).astype(bf16)
    scale = np.float32(1.0 / np.sqrt(DH))
    in_maps = []
    for c in range(NC):
        sl = slice(128 * c, 128 * (c + 1))
        in_maps.append({
            "xT": xT,
            "wq": np.ascontiguousarray((scale * Wq[sl, :]).T).astype(bf16),
            "wk": np.ascontiguousarray(Wk[sl, :].T).astype(bf16),
            "wv": np.ascontiguousarray(Wv[sl, :].T).astype(bf16),
            "wo": np.ascontiguousarray(Wo[:, sl].T).astype(bf16),
            "bqk": np.ascontiguousarray(np.concatenate(
                [scale * bq[sl], bk[sl]]).astype(np.float32)[None, :]),
        })
    return in_maps


def kernel(x, Wq, bq, Wk, bk, Wv, bv, Wo, bo):
    x = np.asarray(x, np.float32)
    Wq, bq = np.asarray(Wq, np.float32), np.asarray(bq, np.float32)
    Wk, bk = np.asarray(Wk, np.float32), np.asarray(bk, np.float32)
    Wv, bv = np.asarray(Wv, np.float32), np.asarray(bv, np.float32)
    Wo, bo = np.asarray(Wo, np.float32), np.asarray(bo, np.float32)

    nc = _build()
    in_maps = _prep_in_maps(x, Wq, bq, Wk, bk, Wv, Wo)
    res = bass_utils.run_bass_kernel_spmd(nc, in_maps, core_ids=list(range(NC)))

    acc = np.zeros((D, T), np.float64)
    for c in range(NC):
        acc += np.asarray(res.results[c]["outT"], np.float64)
    # v-bias folds through softmax (rows sum to 1): + bv @ Wo.T; plus bo.
    const = bo.astype(np.float64) + bv.astype(np.float64) @ Wo.T.astype(np.float64)
    out = acc.T + const[None, :]
    return out.astype(np.float32).reshape(B, S, D)


# revision 29
# speedup vs baseline: 1.1629x; 1.1629x over previous
import sys

if "/opt/trn_rl_repo" not in sys.path:
    sys.path.insert(0, "/opt/trn_rl_repo")

import numpy as np

import concourse.bacc as bacc
import concourse.tile as tile
from concourse import bass_utils, mybir
from concourse.bass import ts
from concourse.masks import make_identity

F32 = mybir.dt.float32
BF16 = mybir.dt.bfloat16
EXP = mybir.ActivationFunctionType.Exp


# nn_MultiHeadedAttention: B=2, S=2048, D=1024, H=16, DH=64.
# 16 heads over 8 cores (2 heads/core = 128 features). QKV column-parallel,
# out-projection row-parallel, host sums the 8 partial outputs.
#
# Schedule: phase-1 chunk 0 runs first, then attention for query-chunk 0
# starts immediately while phase-1 chunks 1..7 are interleaved in small
# units between attention j-iterations (just-in-time for the key tiles
# they produce). Batch-0 out-projections are deferred into batch-1's
# ACT-slack stretches so the scalar engine (softmax exp, the pacing
# engine at ~1.1us per j) never starves. DMA triggers are minimized --
# the sync engine serializes them at ~0.6us apiece.
B, S, D, H = 2, 2048, 1024, 16
DH = D // H
NC = 8
T = B * S                  # 4096 tokens
NCHUNK = T // 512          # 8 token chunks of 512
KCH = D // 128             # 8 contraction chunks
NJ = S // 128              # 16 key tiles per batch
QC = S // 512              # 4 query chunks per batch

_CACHE = {}


def _build():
    if "nc" in _CACHE:
        return _CACHE["nc"]

    nc = bacc.Bacc("TRN2", target_bir_lowering=False, debug=False,
                   enable_asserts=True, num_devices=NC)

    xT = nc.dram_tensor("xT", [D, T], BF16, kind="ExternalInput").ap()
    wq = nc.dram_tensor("wq", [D, 128], BF16, kind="ExternalInput").ap()
    wk = nc.dram_tensor("wk", [D, 128], BF16, kind="ExternalInput").ap()
    wv = nc.dram_tensor("wv", [D, 128], BF16, kind="ExternalInput").ap()
    wo = nc.dram_tensor("wo", [128, D], BF16, kind="ExternalInput").ap()
    bqk = nc.dram_tensor("bqk", [1, 256], F32, kind="ExternalInput").ap()
    outT = nc.dram_tensor("outT", [D, T], BF16, kind="ExternalOutput").ap()

    with tile.TileContext(nc) as tc:
        with (
            tc.tile_pool(name="wpool", bufs=1) as wpool,
            tc.tile_pool(name="qk", bufs=1) as qk_pool,
            tc.tile_pool(name="vtm", bufs=1) as vtm_pool,
            tc.tile_pool(name="on", bufs=1) as on_pool,
            tc.tile_pool(name="xin", bufs=4) as xin_pool,
            tc.tile_pool(name="vst", bufs=2) as vst_pool,
            tc.tile_pool(name="epool", bufs=6) as epool,
            tc.tile_pool(name="npool", bufs=2) as npool,
            tc.tile_pool(name="ostage", bufs=2) as ostage_pool,
            # PSUM budget (8 banks):
            #   psA 2x[128,1024]f32 = 4 banks (scores double-buffer)
            #   psO 1x[65,1024]f32  = 2 banks (o accumulator)
            #   psM 2x 1-bank tiles = 2 banks (q/k/v proj, v transposes,
            #       out-projection -- phase-1 is bank-serial: q releases
            #       its bank before v needs it)
            tc.tile_pool(name="psA", bufs=2, space="PSUM") as psA,
            tc.tile_pool(name="psO", bufs=1, space="PSUM") as psO,
            tc.tile_pool(name="psM", bufs=2, space="PSUM") as psM,
        ):
            # ---- weights / constants ----
            # wq/wk are split into halves (separate tiles, separate DMA
            # queues): per-queue DMA bandwidth is ~50-100 GB/s, and the
            # first projection matmul is gated on the first weight slice.
            wq_h = [wpool.tile([128, D // 2], BF16, name=f"wq{h}")
                    for h in range(2)]
            wk_h = [wpool.tile([128, D // 2], BF16, name=f"wk{h}")
                    for h in range(2)]
            wv_h = [wpool.tile([128, D // 2], BF16, name=f"wv{h}")
                    for h in range(2)]
            wo_sb = wpool.tile([128, D], BF16)

            def wslice(wh, k):
                return wh[k // 4][:, ts(k % 4, 128)]
            bq_sb = wpool.tile([128, 1], F32)
            bk_sb = wpool.tile([128, 1], F32)
            ident = wpool.tile([128, 128], BF16)

            # qn/kn/on persistent activations
            qn = [qk_pool.tile([128, 512], BF16, name=f"qn{n}")
                  for n in range(NCHUNK)]
            kn = [qk_pool.tile([128, 512], BF16, name=f"kn{n}")
                  for n in range(NCHUNK)]
            on = [on_pool.tile([128, 512], BF16, name=f"on{n}")
                  for n in range(NCHUNK)]
            v_tm = {}
            for hh in range(2):
                for J in range(2 * NJ):
                    v_tm[(hh, J)] = vtm_pool.tile(
                        [128, 128], BF16, name=f"vtm{hh}_{J}")

            # ---- phase 1 units for chunk n (emitted piecemeal) ----
            # x for a chunk loads as two tiles (two DMA triggers) so the
            # first projection matmuls can start on the first half.
            def make_ph1(n, nsplit=2):
                st = {}
                kper = KCH // nsplit

                def u_dma():
                    st["xp"] = []
                    for h in range(nsplit):
                        xh = xin_pool.tile([128, kper * 512], BF16, tag="xp",
                                           name=f"xp{n}_{h}")
                        st["xp"].append(xh)
                        nc.sync.dma_start(
                            xh[:],
                            xT[h * kper * 128:(h + 1) * kper * 128,
                               ts(n, 512)].rearrange(
                                "(k p) t -> p k t", p=128))

                def xs(k):
                    return st["xp"][k // kper][:, ts(k % kper, 512)]

                def u_q():
                    st["q_ps"] = psM.tile([128, 512], F32, tag="M",
                                          name=f"qps{n}")
                    for k in range(KCH):
                        nc.tensor.matmul(st["q_ps"][:], wslice(wq_h, k),
                                         xs(k),
                                         start=(k == 0), stop=(k == KCH - 1))

                def u_k():
                    st["k_ps"] = psM.tile([128, 512], F32, tag="M",
                                          name=f"kps{n}")
                    for k in range(KCH):
                        nc.tensor.matmul(st["k_ps"][:], wslice(wk_h, k),
                                         xs(k),
                                         start=(k == 0), stop=(k == KCH - 1))
                    nc.vector.tensor_scalar_add(qn[n][:], st["q_ps"][:],
                                                bq_sb[:])

                def u_v():
                    st["v_ps"] = psM.tile([128, 512], F32, tag="M",
                                          name=f"vps{n}")
                    for k in range(KCH):
                        nc.tensor.matmul(st["v_ps"][:], wslice(wv_h, k),
                                         xs(k),
                                         start=(k == 0), stop=(k == KCH - 1))
                    nc.vector.tensor_scalar_add(kn[n][:], st["k_ps"][:],
                                                bk_sb[:])

                def u_t():
                    vst = vst_pool.tile([128, 512], BF16, tag="vst",
                                        name=f"vst{n}")
                    nc.vector.tensor_copy(vst[:], st["v_ps"][:])
                    for jj in range(4):
                        # one fused [128,128] transpose covers both heads
                        t_ps = psM.tile([128, 1024], BF16, tag="M",
                                        name=f"tps{n}_{jj}")
                        nc.tensor.transpose(t_ps[:, 0:128],
                                            vst[:, ts(jj, 128)], ident[:])
                        # ones column FIRST (sums land on o_ps
                        # partition 0, readable by reciprocal straight
                        # from PSUM); dh at columns 64-127 so the oc copy
                        # reads a 64-aligned partition range. Columns
                        # 1-63 are never read downstream.
                        for hh in range(2):
                            vt = v_tm[(hh, 4 * n + jj)]
                            nc.vector.tensor_copy(
                                vt[:, 64:128], t_ps[:, hh * 64:(hh + 1) * 64])
                            nc.vector.memset(vt[:, 0:1], 1.0)

                return [u_dma, u_q, u_k, u_v, u_t]

            # ---- normalization + out-projection ----
            nstate = {}

            def emit_norm_copy(n, o_ps):
                # sums row sits at o_ps partition 0 (ones column first in
                # v_tm), so reciprocal reads PSUM directly -- no staging
                # copy of the sums row.
                oc = npool.tile([64, 1024], F32, tag="oc", name=f"oc{n}")
                r_sb = npool.tile([1, 1024], F32, tag="r", name=f"r{n}")
                nc.vector.reciprocal_approx_fast(r_sb[:], o_ps[0:1, :])
                nc.vector.tensor_copy(oc[:], o_ps[64:128, :])
                nstate[n] = (oc, r_sb)

            def emit_norm_rest(n):
                oc, r_sb = nstate.pop(n)
                rb = npool.tile([64, 1024], F32, tag="rb", name=f"rb{n}")
                nc.gpsimd.partition_broadcast(rb[:], r_sb[0:1, :])
                for hh in range(2):
                    hs = slice(hh * 64, (hh + 1) * 64)
                    nc.vector.tensor_tensor(
                        out=on[n][hs, :], in0=oc[0:64, ts(hh, 512)],
                        in1=rb[:, ts(hh, 512)], op=mybir.AluOpType.mult)

            def emit_oproj(n):
                for g in range(2):
                    ost = ostage_pool.tile([128, 2048], BF16, tag="ost",
                                           name=f"ost{n}_{g}")
                    for mi in range(4):
                        m = g * 4 + mi
                        op_ps = psM.tile([128, 512], F32, tag="M",
                                         name=f"opps{n}_{m}")
                        nc.tensor.matmul(op_ps[:], wo_sb[:, ts(m, 128)],
                                         on[n][:], start=True, stop=True)
                        nc.vector.tensor_copy(ost[:, ts(mi, 512)],
                                              op_ps[:])
                    # one DMA trigger per 4 m-chunks
                    dst = outT[g * 512:(g + 1) * 512, ts(n, 512)]
                    nc.sync.dma_start(
                        dst.rearrange("(m p) t -> p m t", p=128),
                        ost[:])

            # ---- attention for one query chunk, with hooks ----
            def emit_attn(b, qc, hooks):
                n = b * QC + qc
                o_ps = psO.tile([128, 1024], F32, tag="O", name=f"ops{n}")
                e_prev = None
                for j in range(NJ):
                    s_ps = psA.tile([128, 1024], F32, tag="A",
                                    name=f"sps{n}_{j}")
                    for hh in range(2):
                        hs = slice(hh * 64, (hh + 1) * 64)
                        nc.tensor.matmul(
                            s_ps[:, ts(hh, 512)],
                            kn[b * QC + j // 4][hs, ts(j % 4, 128)],
                            qn[n][hs, :], start=True, stop=True)
                    e_sb = epool.tile([128, 1024], BF16, tag="e",
                                      name=f"e{n}_{j}")
                    nc.scalar.activation(e_sb[:], s_ps[:], EXP)
                    if j >= 1:
                        for hh in range(2):
                            nc.tensor.matmul(
                                o_ps[0:128, ts(hh, 512)],
                                v_tm[(hh, b * NJ + j - 1)][:],
                                e_prev[:, ts(hh, 512)],
                                start=(j - 1 == 0), stop=False)
                    e_prev = e_sb
                    for fn in hooks.get(j, ()):
                        fn()
                for hh in range(2):
                    nc.tensor.matmul(
                        o_ps[0:128, ts(hh, 512)],
                        v_tm[(hh, b * NJ + NJ - 1)][:],
                        e_prev[:, ts(hh, 512)],
                        start=False, stop=True)
                return (n, o_ps)

            # ---- emission schedule ----
            ph1 = [make_ph1(n) for n in range(NCHUNK)]

            # head: identity first (gpsimd, no DMA) so warm-up
            # matmuls can start immediately; then x quarters interleaved
            # with weight halves by first-use order. Biases arrive as one
            # contiguous [1,256] row (a [128,1] DMA costs ~3us of sync
            # descriptor generation) and are scattered to partitions with
            # a K=1 matmul.
            make_identity(nc, ident[:])
            nc.sync.dma_start(
                wq_h[0][:],
                wq[0:512, :].rearrange("(k p) f -> p k f", p=128))
            ph1[0][0]()          # chunk 0 x DMA (2 triggers)
            bqk_sb = wpool.tile([1, 256], F32)
            nc.sync.dma_start(bqk_sb[:], bqk[:])
            nc.sync.dma_start(
                wq_h[1][:],
                wq[512:1024, :].rearrange("(k p) f -> p k f", p=128))
            for h in range(2):
                nc.sync.dma_start(
                    wk_h[h][:],
                    wk[h * 512:(h + 1) * 512, :].rearrange(
                        "(k p) f -> p k f", p=128))
            for h in range(2):
                nc.sync.dma_start(
                    wv_h[h][:],
                    wv[h * 512:(h + 1) * 512, :].rearrange(
                        "(k p) f -> p k f", p=128))
            # Warm the PE's HAM clock gate during the initial DMA wait:
            # ~3.4us of sustained activity lifts the PE from 1.2 to
            # 2.4 GHz, so chunk 0's projections run at full rate.
            warm = [psM.tile([128, 512], F32, tag="M", name=f"warm{i}")
                    for i in range(2)]
            one_sb = wpool.tile([1, 1], F32)
            nc.vector.memset(one_sb[:], 1.0)
            for i in range(40):
                nc.tensor.matmul(warm[i % 2][:, 0:128], ident[:], ident[:],
                                 start=True, stop=True)
            ph1[0][1]()          # u_q
            # scatter biases [1,256] -> [128,1] each via K=1 matmuls
            # (emitted after u_q so the PE never idles waiting for bqk,
            # and before u_k whose bias-add reads bq_sb)
            nc.tensor.matmul(warm[1][:, 0:1], bqk_sb[0:1, 0:128],
                             one_sb[:], start=True, stop=True)
            nc.tensor.matmul(warm[1][:, 1:2], bqk_sb[0:1, 128:256],
                             one_sb[:], start=True, stop=True)
            nc.vector.tensor_copy(bq_sb[:], warm[1][:, 0:1])
            nc.vector.tensor_copy(bk_sb[:], warm[1][:, 1:2])
            ph1[0][2]()          # u_k
            nc.sync.dma_start(wo_sb[:], wo[:])
            # Warm the ACT exp table.
            dummy = wpool.tile([1, 2], F32)
            nc.vector.memset(dummy[:], 0.0)
            nc.scalar.activation(dummy[:], dummy[:], EXP)
            ph1[0][3]()          # u_v
            ph1[0][4]()          # u_t
            ph1[1][0]()          # chunk 1 DMA prefetch

            # stretch 1: qc0 carries ph1 chunks 1-3 (just-in-time for its
            # own key tiles)
            pending = emit_attn(0, 0, {
                0: [ph1[1][1]], 1: [ph1[1][2]], 2: [ph1[1][3]],
                3: [ph1[1][4], ph1[2][0]],
                4: [ph1[2][1]], 5: [ph1[2][2]], 6: [ph1[2][3]],
                7: [ph1[2][4], ph1[3][0]],
                8: [ph1[3][1]], 9: [ph1[3][2]], 10: [ph1[3][3]],
                11: [ph1[3][4]],
            })

            def stretch(b, qc, c=None, defer=None, cur=None, defer2=None):
                """One attention stretch: pending norm (copy at j0, rest
                at j5), optional phase-1 chunk c spread at js 1..12,
                optional deferred out-projections at j3/j12, current
                chunk's out-projection at j9."""
                nonlocal pending
                hooks = {}
                pn, po = pending
                hooks[0] = [lambda: emit_norm_copy(pn, po)]
                if defer is not None:
                    hooks.setdefault(3, []).append(
                        lambda: emit_oproj(defer))
                hooks.setdefault(5, []).append(lambda: emit_norm_rest(pn))
                if cur is not None:
                    hooks.setdefault(9, []).append(
                        lambda: emit_oproj(cur))
                if defer2 is not None:
                    hooks.setdefault(12, []).append(
                        lambda: emit_oproj(defer2))
                if c is not None:
                    hooks.setdefault(1, []).append(ph1[c][0])
                    hooks.setdefault(3, []).append(ph1[c][1])
                    hooks.setdefault(6, []).append(ph1[c][2])
                    hooks.setdefault(9, []).append(ph1[c][3])
                    hooks.setdefault(12, []).append(ph1[c][4])
                pending = emit_attn(b, qc, hooks)

            stretch(0, 1, c=4)
            stretch(0, 2, c=5)
            stretch(0, 3, c=6)
            stretch(1, 0, c=7)
            stretch(1, 1, defer=0, cur=4)
            stretch(1, 2, defer=1, cur=5)
            stretch(1, 3, defer=2, cur=6, defer2=3)

            # tail: chunk 7 norm + out-projection. The on[] multiplies
            # read o_ps straight from PSUM (no oc staging copy -- nothing
            # else needs the banks), and dummy matmuls keep the PE's HAM
            # clock warm through the ~4us norm chain so the final
            # out-projection runs at full rate.
            pn, po = pending          # chunk 7
            r_sb = npool.tile([1, 1024], F32, tag="r", name=f"r{pn}")
            nc.vector.reciprocal_approx_fast(r_sb[:], po[0:1, :])
            rb = npool.tile([64, 1024], F32, tag="rb", name=f"rb{pn}")
            nc.gpsimd.partition_broadcast(rb[:], r_sb[0:1, :])
            wtl = [psM.tile([128, 512], F32, tag="M", name=f"wtl{i}")
                   for i in range(2)]
            for i in range(16):
                nc.tensor.matmul(wtl[i % 2][:, 0:256],
                                 ident[:], qn[0][:, 0:256],
                                 start=True, stop=True)
            for hh in range(2):
                hs = slice(hh * 64, (hh + 1) * 64)
                nc.vector.tensor_tensor(
                    out=on[pn][hs, :], in0=po[64:128, ts(hh, 512)],
                    in1=rb[:, ts(hh, 512)], op=mybir.AluOpType.mult)
            for i in range(8):
                nc.tensor.matmul(wtl[i % 2][:, 0:256],
                                 ident[:], qn[0][:, 0:256],
                                 start=True, stop=True)
            emit_oproj(pn)

    nc.compile()
    _CACHE["nc"] = nc
    return nc


def _prep_in_maps(x, Wq, bq, Wk, bk, Wv, Wo):
    import ml_dtypes
    bf16 = ml_dtypes.bfloat16
    xT = np.ascontiguousarray(x.reshape(T, D).T).astype(bf16)
    scale = np.float32(1.0 / np.sqrt(DH))
    in_maps = []
    for c in range(NC):
        sl = slice(128 * c, 128 * (c + 1))
        in_maps.append({
            "xT": xT,
            "wq": np.ascontiguousarray((scale * Wq[sl, :]).T).astype(bf16),
            "wk": np.ascontiguousarray(Wk[sl, :].T).astype(bf16),
            "wv": np.ascontiguousarray(Wv[sl, :].T).astype(bf16),
            "wo": np.ascontiguousarray(Wo[:, sl].T).astype(bf16),
            "bqk": np.ascontiguousarray(np.concatenate(
                [scale * bq[sl], bk[sl]]).astype(np.float32)[None, :]),
        })
    return in_maps


def kernel(x, Wq, bq, Wk, bk, Wv, bv, Wo, bo):
    x = np.asarray(x, np.float32)
    Wq, bq = np.asarray(Wq, np.float32), np.asarray(bq, np.float32)
    Wk, bk = np.asarray(Wk, np.float32), np.asarray(bk, np.float32)
    Wv, bv = np.asarray(Wv, np.float32), np.asarray(bv, np.float32)
    Wo, bo = np.asarray(Wo, np.float32), np.asarray(bo, np.float32)

    nc = _build()
    in_maps = _prep_in_maps(x, Wq, bq, Wk, bk, Wv, Wo)
    res = bass_utils.run_bass_kernel_spmd(nc, in_maps, core_ids=list(range(NC)))

    acc = np.zeros((D, T), np.float64)
    for c in range(NC):
        acc += np.asarray(res.results[c]["outT"], np.float64)
    # v-bias folds through softmax (rows sum to 1): + bv @ Wo.T; plus bo.
    const = bo.astype(np.float64) + bv.astype(np.float64) @ Wo.T.astype(np.float64)
    out = acc.T + const[None, :]
    return out.astype(np.float32).reshape(B, S, D)
